# revision 7
# baseline (speedup 1.0000x reference)
"""Trainium2 Bass kernel for nn_CrossAttention: two-stream (rgb/depth) cross
attention, B=8 batch elements data-parallel across 8 NeuronCores.

Per core (one batch element b):
  rgb = x[:1024], depth = x[1024:]
  qkv_m = rgb/depth @ W_m + b_m          (H=8 heads, D=64)
  rgb_out   = softmax(q_dep k_rgb^T / 8) v_rgb   -> out tokens    0..1023
  depth_out = softmax(q_rgb k_dep^T / 8) v_dep   -> out tokens 1024..2047
  out = concat @ W_proj + b_proj

Layout strategy (all matmuls float32r: full-rate PE with ~1e-4 rel err):
  - x is PE-transposed to xT[c, tok] once.
  - Q,K produced feature-major (qkT[feat, tok]) so scores^T = k^T q needs no
    further transpose; V produced token-major with a ones-column appended so
    the attn@v matmul also emits the softmax denominator Z as row 64.
  - softmax runs unnormalized (scores ~ N(0,1): exp never overflows);
    normalization happens at the attention-output evac via 1/Z broadcast.
  - attention output is d-major = exactly the lhsT the final projection needs.
"""
import numpy as np

import concourse.bass as bass
import concourse.mybir as mybir
import concourse.tile as tile
from concourse import bacc
from concourse.bass_utils import run_bass_kernel_spmd
from concourse.bass_interp import get_hw_module

f32 = mybir.dt.float32
f32r = mybir.dt.float32r
AF = mybir.ActivationFunctionType
Alu = mybir.AluOpType

N_CORES = 8
B, N, C = 8, 2048, 512
H, D, L = 8, 64, 1024
SCALE = float(D) ** -0.5


def _emit(nc, tc, x, w_qkv, b_qkv, w_proj, b_proj, out):
    from concourse.masks import make_identity

    with (
        tc.tile_pool(name="persist", bufs=1) as persist,
        tc.tile_pool(name="work", bufs=1) as work,
    ):
        # ---------- persistent tensors ----------
        qkT = [persist.tile([128, 8, 1024], f32r, name=f"qkT{m}") for m in range(2)]
        vaug = [persist.tile([128, 8, 8, 65], f32r, name=f"vaug{m}") for m in range(2)]
        wp = persist.tile([128, 4, 512], f32r, name="wp")
        nc.gpsimd.dma_start(out=wp[:, :, :],
                            in_=w_proj.rearrange("(k p) f -> p k f", p=128))
        bqk = []
        bv = []
        for m in range(2):
            bq = persist.tile([128, 8], f32, name=f"bqk{m}")
            nc.sync.dma_start(out=bq[:, :],
                              in_=b_qkv[m][0:1024].rearrange("(t p) -> p t", p=128))
            bqk.append(bq)
            bv1 = persist.tile([1, 512], f32, name=f"bv1_{m}")
            nc.sync.dma_start(out=bv1[:, :],
                              in_=b_qkv[m][1024:1536].rearrange("(o f) -> o f", o=1))
            bvm = persist.tile([128, 512], f32, name=f"bv{m}")
            nc.gpsimd.partition_broadcast(bvm[:, :], bv1[:, :])
            bv.append(bvm)
        bp1 = persist.tile([1, 512], f32, name="bp1")
        nc.sync.dma_start(out=bp1[:, :], in_=b_proj.rearrange("(o f) -> o f", o=1))
        bp = persist.tile([128, 512], f32, name="bp")
        nc.gpsimd.partition_broadcast(bp[:, :], bp1[:, :])

        ones64 = persist.tile([128, 64], f32, name="ones64")
        nc.gpsimd.memset(ones64[:, :], 1.0)
        for m in range(2):
            nc.vector.tensor_copy(
                vaug[m][:, :, :, 64:65],
                ones64.rearrange("p (c h o) -> p c h o", c=8, h=8))

        # ---------- phase 1: x transpose + qkv ----------
        with (
            tc.tile_pool(name="qkvp", bufs=1) as qkvp,
            tc.tile_pool(name="psA", bufs=1, space="PSUM") as psA,
        ):
            ident = qkvp.tile([128, 128], f32, name="ident")
            make_identity(nc, ident[:, :])
            xT = qkvp.tile([128, 4, 2048], f32r, name="xT")
            wqk = []
            wv = []
            for m in range(2):
                wq = qkvp.tile([128, 4, 1024], f32r, name=f"wqk{m}")
                nc.gpsimd.dma_start(
                    out=wq[:, :, :],
                    in_=w_qkv[m][:, 0:1024].rearrange("(k p) f -> p k f", p=128))
                wqk.append(wq)
                wvm = qkvp.tile([128, 4, 512], f32r, name=f"wv{m}")
                nc.gpsimd.dma_start(
                    out=wvm[:, :, :],
                    in_=w_qkv[m][:, 1024:1536].rearrange("(k p) f -> p k f", p=128))
                wv.append(wvm)

            with nc.named_scope("transpose"):
                for ti in range(16):
                    xsrc = qkvp.tile([128, 512], f32, name=f"xsrc{ti}",
                                     tag="xsrc", bufs=2)
                    nc.sync.dma_start(out=xsrc[:, :],
                                      in_=x[ti * 128:(ti + 1) * 128, :])
                    for k in range(4):
                        tps = psA.tile([128, 128], f32, name=f"tp{ti}_{k}",
                                       tag="tp", bufs=2)
                        nc.tensor.transpose(tps[:, :],
                                            xsrc[:, k * 128:(k + 1) * 128],
                                            ident[:, :])
                        nc.vector.tensor_copy(
                            xT[:, k, ti * 128:(ti + 1) * 128], tps[:, :])

            with nc.named_scope("qk"):
                for m in range(2):
                    for j in range(8):
                        qk_ps = psA.tile([128, 1024], f32, name=f"qk{m}_{j}",
                                         tag="qk", bufs=2)
                        for th in range(2):
                            for k in range(4):
                                nc.tensor.matmul(
                                    qk_ps[:, th * 512:(th + 1) * 512],
                                    wqk[m][:, k, j * 128:(j + 1) * 128],
                                    xT[:, k, m * 1024 + th * 512:
                                       m * 1024 + (th + 1) * 512],
                                    start=(k == 0), stop=(k == 3))
                        nc.vector.tensor_scalar(
                            out=qkT[m][:, j, :], in0=qk_ps[:, :],
                            scalar1=bqk[m][:, j:j + 1], scalar2=None,
                            op0=Alu.add)

            with nc.named_scope("v"):
                for m in range(2):
                    for c in range(8):
                        v_ps = psA.tile([128, 512], f32, name=f"v{m}_{c}",
                                        tag="v", bufs=2)
                        for k in range(4):
                            nc.tensor.matmul(
                                v_ps[:, :],
                                xT[:, k, m * 1024 + c * 128:
                                   m * 1024 + (c + 1) * 128],
                                wv[m][:, k, :],
                                start=(k == 0), stop=(k == 3))
                        nc.vector.tensor_tensor(
                            out=vaug[m][:, c, :, 0:64],
                            in0=v_ps.rearrange("p (h d) -> p h d", h=8),
                            in1=bv[m].rearrange("p (h d) -> p h d", h=8),
                            op=Alu.add)

        # ---------- phase 2: cross attention ----------
        # units: u in [0,128) per dir; head h=u//16, chunk c=(u%16)//2, qh=u%2
        # psum scores tiles pack 3 units (ACT exp runs on [128,1536] tiles)
        with tc.tile_pool(name="att2", bufs=1) as att2:
            oT = [att2.tile([128, 4, 1024], f32r, name=f"oT{d}") for d in range(2)]
            with (
                tc.tile_pool(name="attp", bufs=1) as attp,
                tc.tile_pool(name="psB", bufs=1, space="PSUM") as psB,
                nc.named_scope("attention"),
            ):
                for d in range(2):
                    qm, kvm = 1 - d, d
                    oT_ps = None
                    u = 0
                    while u < 128:
                        n_u = min(3, 128 - u)
                        s_ps = psB.tile([128, n_u * 512], f32,
                                        name=f"s{d}_{u}", tag="sc", bufs=2,
                                        padded_shape=[128, 1536])
                        for i in range(n_u):
                            h, c, qh = (u + i) // 16, ((u + i) % 16) // 2, (u + i) % 2
                            nc.tensor.matmul(
                                s_ps[:, i * 512:(i + 1) * 512],
                                qkT[kvm][(h % 2) * 64:(h % 2) * 64 + 64,
                                         4 + h // 2, c * 128:(c + 1) * 128],
                                qkT[qm][(h % 2) * 64:(h % 2) * 64 + 64,
                                        h // 2, qh * 512:(qh + 1) * 512],
                                start=True, stop=True)
                        exp_t = attp.tile([128, n_u * 512], f32r,
                                          name=f"e{d}_{u}", tag="exp", bufs=4,
                                          padded_shape=[128, 1536])
                        nc.scalar.activation(exp_t[:, :], s_ps[:, :],
                                             AF.Exp, scale=SCALE)
                        for i in range(n_u):
                            h, c, qh = (u + i) // 16, ((u + i) % 16) // 2, (u + i) % 2
                            if (u + i) % 16 == 0:
                                oT_ps = psB.tile([65, 1024], f32,
                                                 name=f"o{d}_{h}", tag="oT",
                                                 bufs=1)
                            nc.tensor.matmul(
                                oT_ps[:, qh * 512:(qh + 1) * 512],
                                vaug[kvm][:, c, h, :],
                                exp_t[:, i * 512:(i + 1) * 512],
                                start=(c == 0), stop=(c == 7))
                            if (u + i) % 16 == 15:
                                rz = attp.tile([1, 1024], f32,
                                               name=f"rz{d}_{h}", tag="rz",
                                               bufs=2)
                                nc.vector.reciprocal(rz[:, :], oT_ps[64:65, :])
                                rzb = attp.tile([64, 1024], f32,
                                                name=f"rzb{d}_{h}", tag="rzb",
                                                bufs=2)
                                nc.gpsimd.partition_broadcast(rzb[:, :], rz[:, :])
                                nc.vector.tensor_tensor(
                                    out=oT[d][(h % 2) * 64:(h % 2) * 64 + 64,
                                              h // 2, :],
                                    in0=oT_ps[0:64, :], in1=rzb[:, :],
                                    op=Alu.mult)
                        u += n_u

            # ---------- phase 3: output projection (inside att2: reads oT) ----
            with tc.tile_pool(name="psC", bufs=1, space="PSUM") as psC, \
                 nc.named_scope("proj"):
                for d in range(2):
                    for tt in range(8):
                        pj_ps = psC.tile([128, 512], f32, name=f"pj{d}_{tt}",
                                         tag="pj", bufs=3)
                        for g in range(4):
                            nc.tensor.matmul(
                                pj_ps[:, :],
                                oT[d][:, g, tt * 128:(tt + 1) * 128],
                                wp[:, g, :],
                                start=(g == 0), stop=(g == 3))
                        ost = work.tile([128, 512], f32, name=f"ost{d}_{tt}",
                                        tag="ost", bufs=3)
                        nc.vector.tensor_tensor(out=ost[:, :], in0=pj_ps[:, :],
                                                in1=bp[:, :], op=Alu.add)
                        nc.sync.dma_start(
                            out=out[d * 1024 + tt * 128:
                                    d * 1024 + (tt + 1) * 128, :],
                            in_=ost[:, :])


def build_module():
    nc = bacc.Bacc("TRN2", target_bir_lowering=False, debug=False,
                   num_devices=N_CORES)
    x = nc.dram_tensor("x", [N, C], f32, kind="ExternalInput").ap()
    w_rgb = nc.dram_tensor("w_rgb", [C, 3 * C], f32, kind="ExternalInput").ap()
    b_rgb = nc.dram_tensor("b_rgb", [3 * C], f32, kind="ExternalInput").ap()
    w_dep = nc.dram_tensor("w_dep", [C, 3 * C], f32, kind="ExternalInput").ap()
    b_dep = nc.dram_tensor("b_dep", [3 * C], f32, kind="ExternalInput").ap()
    w_proj = nc.dram_tensor("w_proj", [C, C], f32, kind="ExternalInput").ap()
    b_proj = nc.dram_tensor("b_proj", [C], f32, kind="ExternalInput").ap()
    out = nc.dram_tensor("out", [N, C], f32, kind="ExternalOutput").ap()

    with tile.TileContext(nc) as tc:
        _emit(nc, tc, x, [w_rgb, w_dep], [b_rgb, b_dep], w_proj, b_proj, out)
    nc.compile()
    nc.m = get_hw_module(nc.m)
    return nc


_NC_CACHE = None


def kernel(x, W_rgb_qkv, b_rgb_qkv, W_depth_qkv, b_depth_qkv, W_proj, b_proj):
    global _NC_CACHE
    if _NC_CACHE is None:
        _NC_CACHE = build_module()
    nc = _NC_CACHE

    x = np.ascontiguousarray(np.asarray(x, dtype=np.float32))
    shared = {
        "w_rgb": np.ascontiguousarray(np.asarray(W_rgb_qkv, np.float32)),
        "b_rgb": np.ascontiguousarray(np.asarray(b_rgb_qkv, np.float32)),
        "w_dep": np.ascontiguousarray(np.asarray(W_depth_qkv, np.float32)),
        "b_dep": np.ascontiguousarray(np.asarray(b_depth_qkv, np.float32)),
        "w_proj": np.ascontiguousarray(np.asarray(W_proj, np.float32)),
        "b_proj": np.ascontiguousarray(np.asarray(b_proj, np.float32)),
    }
    in_maps = [{"x": x[i], **shared} for i in range(N_CORES)]
    res = run_bass_kernel_spmd(nc, in_maps, core_ids=list(range(N_CORES)))
    return np.stack([res.results[i]["out"] for i in range(N_CORES)], axis=0)


# revision 9
# speedup vs baseline: 1.5357x; 1.5357x over previous
"""Trainium2 Bass kernel for nn_CrossAttention: two-stream (rgb/depth) cross
attention, B=8 batch elements data-parallel across 8 NeuronCores.

Per core (one batch element b):
  rgb = x[:1024], depth = x[1024:]
  qkv_m = rgb/depth @ W_m + b_m          (H=8 heads, D=64)
  rgb_out   = softmax(q_dep k_rgb^T / 8) v_rgb   -> out tokens    0..1023
  depth_out = softmax(q_rgb k_dep^T / 8) v_dep   -> out tokens 1024..2047
  out = concat @ W_proj + b_proj

Layout strategy (all matmuls float32r: full-rate PE with ~1e-4 rel err):
  - x is PE-transposed to xT[c, tok] once.
  - Q,K produced feature-major (qkT[feat, tok]) so scores^T = k^T q needs no
    further transpose; V produced token-major with a ones-column appended so
    the attn@v matmul also emits the softmax denominator Z as row 64.
  - softmax runs unnormalized (scores ~ N(0,1): exp never overflows);
    normalization happens at the attention-output evac via 1/Z broadcast.
  - attention output is d-major = exactly the lhsT the final projection needs.
"""
import numpy as np

import concourse.bass as bass
import concourse.mybir as mybir
import concourse.tile as tile
from concourse import bacc
from concourse.bass_utils import run_bass_kernel_spmd
from concourse.bass_interp import get_hw_module

f32 = mybir.dt.float32
f32r = mybir.dt.float32r
AF = mybir.ActivationFunctionType
Alu = mybir.AluOpType

N_CORES = 8
B, N, C = 8, 2048, 512
H, D, L = 8, 64, 1024
SCALE = float(D) ** -0.5


def _emit(nc, tc, x, w_qkv, b_qkv, w_proj, b_proj, out):
    from concourse.masks import make_identity

    with (
        tc.tile_pool(name="persist", bufs=1) as persist,
        tc.tile_pool(name="work", bufs=1) as work,
    ):
        # ---------- persistent tensors ----------
        qkT = [persist.tile([128, 8, 1024], f32r, name=f"qkT{m}") for m in range(2)]
        vaug = [persist.tile([128, 8, 8, 65], f32r, name=f"vaug{m}") for m in range(2)]
        wp = persist.tile([128, 4, 512], f32r, name="wp")
        nc.gpsimd.dma_start(out=wp[:, :, :],
                            in_=w_proj.rearrange("(k p) f -> p k f", p=128))
        bqk = []
        bv = []
        for m in range(2):
            bq = persist.tile([128, 8], f32, name=f"bqk{m}")
            nc.sync.dma_start(out=bq[:, :],
                              in_=b_qkv[m][0:1024].rearrange("(t p) -> p t", p=128))
            bqk.append(bq)
            bv1 = persist.tile([1, 512], f32, name=f"bv1_{m}")
            nc.sync.dma_start(out=bv1[:, :],
                              in_=b_qkv[m][1024:1536].rearrange("(o f) -> o f", o=1))
            bvm = persist.tile([128, 512], f32, name=f"bv{m}")
            nc.gpsimd.partition_broadcast(bvm[:, :], bv1[:, :])
            bv.append(bvm)
        bp1 = persist.tile([1, 512], f32, name="bp1")
        nc.sync.dma_start(out=bp1[:, :], in_=b_proj.rearrange("(o f) -> o f", o=1))
        bp = persist.tile([128, 512], f32, name="bp")
        nc.gpsimd.partition_broadcast(bp[:, :], bp1[:, :])

        ones64 = persist.tile([128, 64], f32, name="ones64")
        nc.gpsimd.memset(ones64[:, :], 1.0)
        for m in range(2):
            nc.vector.tensor_copy(
                vaug[m][:, :, :, 64:65],
                ones64.rearrange("p (c h o) -> p c h o", c=8, h=8))

        # ---------- phase 1: x transpose + qkv ----------
        with (
            tc.tile_pool(name="qkvp", bufs=1) as qkvp,
            tc.tile_pool(name="psA", bufs=1, space="PSUM") as psA,
        ):
            ident = qkvp.tile([128, 128], f32, name="ident")
            make_identity(nc, ident[:, :])
            xT = qkvp.tile([128, 4, 2048], f32r, name="xT")
            wqk = []
            wv = []
            for m in range(2):
                wq = qkvp.tile([128, 4, 1024], f32r, name=f"wqk{m}")
                nc.gpsimd.dma_start(
                    out=wq[:, :, :],
                    in_=w_qkv[m][:, 0:1024].rearrange("(k p) f -> p k f", p=128))
                wqk.append(wq)
                wvm = qkvp.tile([128, 4, 512], f32r, name=f"wv{m}")
                nc.gpsimd.dma_start(
                    out=wvm[:, :, :],
                    in_=w_qkv[m][:, 1024:1536].rearrange("(k p) f -> p k f", p=128))
                wv.append(wvm)

            with nc.named_scope("transpose"):
                for ti in range(16):
                    xsrc = qkvp.tile([128, 512], f32, name=f"xsrc{ti}",
                                     tag="xsrc", bufs=2)
                    nc.sync.dma_start(out=xsrc[:, :],
                                      in_=x[ti * 128:(ti + 1) * 128, :])
                    for k in range(4):
                        tps = psA.tile([128, 128], f32, name=f"tp{ti}_{k}",
                                       tag="tp", bufs=2)
                        nc.tensor.transpose(tps[:, :],
                                            xsrc[:, k * 128:(k + 1) * 128],
                                            ident[:, :])
                        nc.vector.tensor_copy(
                            xT[:, k, ti * 128:(ti + 1) * 128], tps[:, :])

            with nc.named_scope("qk"):
                for m in range(2):
                    for j in range(8):
                        qk_ps = psA.tile([128, 1024], f32, name=f"qk{m}_{j}",
                                         tag="qk", bufs=2)
                        for th in range(2):
                            for k in range(4):
                                nc.tensor.matmul(
                                    qk_ps[:, th * 512:(th + 1) * 512],
                                    wqk[m][:, k, j * 128:(j + 1) * 128],
                                    xT[:, k, m * 1024 + th * 512:
                                       m * 1024 + (th + 1) * 512],
                                    start=(k == 0), stop=(k == 3))
                        nc.vector.tensor_scalar(
                            out=qkT[m][:, j, :], in0=qk_ps[:, :],
                            scalar1=bqk[m][:, j:j + 1], scalar2=None,
                            op0=Alu.add)

            with nc.named_scope("v"):
                for m in range(2):
                    for c in range(8):
                        v_ps = psA.tile([128, 512], f32, name=f"v{m}_{c}",
                                        tag="v", bufs=2)
                        for k in range(4):
                            nc.tensor.matmul(
                                v_ps[:, :],
                                xT[:, k, m * 1024 + c * 128:
                                   m * 1024 + (c + 1) * 128],
                                wv[m][:, k, :],
                                start=(k == 0), stop=(k == 3))
                        nc.vector.tensor_tensor(
                            out=vaug[m][:, c, :, 0:64],
                            in0=v_ps.rearrange("p (h d) -> p h d", h=8),
                            in1=bv[m].rearrange("p (h d) -> p h d", h=8),
                            op=Alu.add)

        # ---------- phase 2: cross attention ----------
        # units: u in [0,128) per dir; head h=u//16, chunk c=(u%16)//2, qh=u%2
        # psum scores tiles pack 3 units (ACT exp runs on [128,1536] tiles)
        with tc.tile_pool(name="att2", bufs=1) as att2:
            oT = [att2.tile([128, 4, 1024], f32r, name=f"oT{d}") for d in range(2)]
            with (
                tc.tile_pool(name="attp", bufs=1) as attp,
                tc.tile_pool(name="psB", bufs=1, space="PSUM") as psB,
                nc.named_scope("attention"),
            ):
                for d in range(2):
                    qm, kvm = 1 - d, d
                    oT_ps = None
                    u = 0
                    while u < 128:
                        n_u = min(3, 128 - u)
                        s_ps = psB.tile([128, n_u * 512], f32,
                                        name=f"s{d}_{u}", tag="sc", bufs=2,
                                        padded_shape=[128, 1536])
                        for i in range(n_u):
                            h, c, qh = (u + i) // 16, ((u + i) % 16) // 2, (u + i) % 2
                            nc.tensor.matmul(
                                s_ps[:, i * 512:(i + 1) * 512],
                                qkT[kvm][(h % 2) * 64:(h % 2) * 64 + 64,
                                         4 + h // 2, c * 128:(c + 1) * 128],
                                qkT[qm][(h % 2) * 64:(h % 2) * 64 + 64,
                                        h // 2, qh * 512:(qh + 1) * 512],
                                start=True, stop=True)
                        exp_t = attp.tile([128, n_u * 512], f32r,
                                          name=f"e{d}_{u}", tag="exp", bufs=4,
                                          padded_shape=[128, 1536])
                        nc.scalar.activation(exp_t[:, :], s_ps[:, :],
                                             AF.Exp, scale=SCALE)
                        for i in range(n_u):
                            h, c, qh = (u + i) // 16, ((u + i) % 16) // 2, (u + i) % 2
                            if (u + i) % 16 == 0:
                                oT_ps = psB.tile([65, 1024], f32,
                                                 name=f"o{d}_{h}", tag="oT",
                                                 bufs=1)
                            nc.tensor.matmul(
                                oT_ps[:, qh * 512:(qh + 1) * 512],
                                vaug[kvm][:, c, h, :],
                                exp_t[:, i * 512:(i + 1) * 512],
                                start=(c == 0), stop=(c == 7))
                            if (u + i) % 16 == 15:
                                # fast evac releases the oT psum slot; the
                                # slow 1/Z chain then runs off critical path
                                oTs = attp.tile([65, 1024], f32,
                                                name=f"oTs{d}_{h}", tag="oTs",
                                                bufs=3)
                                nc.vector.tensor_copy(oTs[:, :], oT_ps[:, :])
                                rz = attp.tile([1, 1024], f32,
                                               name=f"rz{d}_{h}", tag="rz",
                                               bufs=2)
                                nc.vector.reciprocal(rz[:, :], oTs[64:65, :])
                                rzb = attp.tile([64, 1024], f32,
                                                name=f"rzb{d}_{h}", tag="rzb",
                                                bufs=2)
                                nc.gpsimd.partition_broadcast(rzb[:, :], rz[:, :])
                                nc.vector.tensor_tensor(
                                    out=oT[d][(h % 2) * 64:(h % 2) * 64 + 64,
                                              h // 2, :],
                                    in0=oTs[0:64, :], in1=rzb[:, :],
                                    op=Alu.mult)
                        u += n_u

            # ---------- phase 3: output projection (inside att2: reads oT) ----
            with tc.tile_pool(name="psC", bufs=1, space="PSUM") as psC, \
                 nc.named_scope("proj"):
                for d in range(2):
                    for tt in range(8):
                        pj_ps = psC.tile([128, 512], f32, name=f"pj{d}_{tt}",
                                         tag="pj", bufs=3)
                        for g in range(4):
                            nc.tensor.matmul(
                                pj_ps[:, :],
                                oT[d][:, g, tt * 128:(tt + 1) * 128],
                                wp[:, g, :],
                                start=(g == 0), stop=(g == 3))
                        ost = work.tile([128, 512], f32, name=f"ost{d}_{tt}",
                                        tag="ost", bufs=3)
                        nc.vector.tensor_tensor(out=ost[:, :], in0=pj_ps[:, :],
                                                in1=bp[:, :], op=Alu.add)
                        nc.sync.dma_start(
                            out=out[d * 1024 + tt * 128:
                                    d * 1024 + (tt + 1) * 128, :],
                            in_=ost[:, :])


def build_module():
    nc = bacc.Bacc("TRN2", target_bir_lowering=False, debug=False,
                   num_devices=N_CORES)
    x = nc.dram_tensor("x", [N, C], f32, kind="ExternalInput").ap()
    w_rgb = nc.dram_tensor("w_rgb", [C, 3 * C], f32, kind="ExternalInput").ap()
    b_rgb = nc.dram_tensor("b_rgb", [3 * C], f32, kind="ExternalInput").ap()
    w_dep = nc.dram_tensor("w_dep", [C, 3 * C], f32, kind="ExternalInput").ap()
    b_dep = nc.dram_tensor("b_dep", [3 * C], f32, kind="ExternalInput").ap()
    w_proj = nc.dram_tensor("w_proj", [C, C], f32, kind="ExternalInput").ap()
    b_proj = nc.dram_tensor("b_proj", [C], f32, kind="ExternalInput").ap()
    out = nc.dram_tensor("out", [N, C], f32, kind="ExternalOutput").ap()

    with tile.TileContext(nc) as tc:
        _emit(nc, tc, x, [w_rgb, w_dep], [b_rgb, b_dep], w_proj, b_proj, out)
    nc.compile()
    nc.m = get_hw_module(nc.m)
    return nc


_NC_CACHE = None


def kernel(x, W_rgb_qkv, b_rgb_qkv, W_depth_qkv, b_depth_qkv, W_proj, b_proj):
    global _NC_CACHE
    if _NC_CACHE is None:
        _NC_CACHE = build_module()
    nc = _NC_CACHE

    x = np.ascontiguousarray(np.asarray(x, dtype=np.float32))
    shared = {
        "w_rgb": np.ascontiguousarray(np.asarray(W_rgb_qkv, np.float32)),
        "b_rgb": np.ascontiguousarray(np.asarray(b_rgb_qkv, np.float32)),
        "w_dep": np.ascontiguousarray(np.asarray(W_depth_qkv, np.float32)),
        "b_dep": np.ascontiguousarray(np.asarray(b_depth_qkv, np.float32)),
        "w_proj": np.ascontiguousarray(np.asarray(W_proj, np.float32)),
        "b_proj": np.ascontiguousarray(np.asarray(b_proj, np.float32)),
    }
    in_maps = [{"x": x[i], **shared} for i in range(N_CORES)]
    res = run_bass_kernel_spmd(nc, in_maps, core_ids=list(range(N_CORES)))
    return np.stack([res.results[i]["out"] for i in range(N_CORES)], axis=0)
